# revision 47
# baseline (speedup 1.0000x reference)
"""MoE expert-network kernel for 8 Trainium2 NeuronCores.

Strategy: expert parallelism (E == n_cores == 8). The host dispatches each
token to its expert's core (an all-to-all in numpy), folds the inference-mode
BatchNorm into the expert weights/bias, and each core runs one dense
[cap, 512] @ [512, 512] GEMM fused with bias + SiLU via the activation engine.

All device tensors are laid out host-side as the exact SBUF tile images
(128-partition-major, block-contiguous per token tile) so every DMA is a
plain 2D contiguous copy with multi-KB lines.

Per-core device program (identical on all cores, SPMD):
  inputs : xs [128, KC*cap]  fp16 - token tiles, partition-major blocks
           ws [128, KC*HID]  fp16 - BN-folded weight tile image
           bs [128, MC]      fp32 - BN-folded bias tile image
  output : os [128, MC*cap]  fp16 - silu(x @ W + b), block per token tile
x is shipped fp16 (~2e-4 rel error, halves the dominant stream); the host
scatters the result back into the full [B, 512] fp32 output.
"""

import sys

for _p in ("/opt/trn_rl_repo",):
    if _p not in sys.path:
        sys.path.append(_p)

import numpy as np

import concourse.bass as bass
import concourse.mybir as mybir
import concourse.tile as tile
from concourse import bacc
from concourse.bass_utils import run_bass_kernel_spmd

B = 32768
IN = 512
HID = 512
E = 8
NCORES = 8
EPS = 1e-5
P = 128  # SBUF partitions
NT = 512  # matmul moving-dim chunk (one fp32 PSUM bank)

KC = IN // P  # contraction chunks
MC = HID // P  # output-feature chunks


def plan_sizes(cap: int) -> list:
    """Token-tile sizes: small tiles at the start (fast pipeline ramp) and at
    the end (short final ACT->store tail), 1024-wide tiles in the middle."""
    sizes = []
    rem = cap
    if rem >= 256 + 1024:
        sizes.append(256)
        rem -= 256
    if rem >= 512 + 1024 + 128:
        sizes.append(512)
        rem -= 512
    while rem >= 1024 + 128:
        sizes.append(1024)
        rem -= 1024
    if rem > 128:
        sizes.append(rem - 128)
        rem = 128
    if rem:
        sizes.append(rem)
    return sizes


def build_bass(cap: int, act: str = "silu") -> bass.Bass:
    nc = bacc.Bacc(
        "TRN2",
        target_bir_lowering=False,
        debug=False,
        enable_asserts=False,
        num_devices=NCORES,
    )
    f32 = mybir.dt.float32
    f16 = mybir.dt.float16

    xs = nc.dram_tensor("xs", [P, KC * cap], f16, kind="ExternalInput").ap()
    ws = nc.dram_tensor("ws", [P, KC * HID], f16, kind="ExternalInput").ap()
    bs = nc.dram_tensor("bs", [P, MC], f32, kind="ExternalInput").ap()
    os_ = nc.dram_tensor("os", [P, MC * cap], f16, kind="ExternalOutput").ap()

    tiles = []
    n0 = 0
    for s in plan_sizes(cap):
        tiles.append((n0, s))
        n0 += s

    with tile.TileContext(nc) as tc:
        with (
            tc.tile_pool(name="wpool", bufs=1) as wpool,
            tc.tile_pool(name="xpool", bufs=10) as xpool,
            tc.tile_pool(name="opool", bufs=8) as opool,
            tc.tile_pool(name="pp", bufs=8, space="PSUM") as pp,
        ):
            # Weight/bias loads ride the scalar HWDGE ring, token loads the
            # sync ring: their triggers issue in parallel after the preamble.
            wt = wpool.tile([P, KC, HID], f16, tag="wt", name="wt")
            nc.scalar.dma_start(out=wt, in_=ws)
            bt = wpool.tile([P, MC], f32, tag="bt", name="bt")
            nc.scalar.dma_start(out=bt, in_=bs)

            for n0, nt in tiles:
                xt = xpool.tile([P, KC, nt], f16, tag="xt", name="xt")
                nc.sync.dma_start(out=xt, in_=xs[:, KC * n0 : KC * (n0 + nt)])
                ot = opool.tile([P, MC, nt], f16, tag="ot", name="ot")
                for off in range(0, nt, NT):
                    ns = min(NT, nt - off)
                    for m in range(MC):
                        ps = pp.tile([P, ns], f32, tag="ps", name="ps")
                        for k in range(KC):
                            nc.tensor.matmul(
                                ps,
                                lhsT=wt[:, k, m * P : (m + 1) * P],
                                rhs=xt[:, k, off : off + ns],
                                start=(k == 0),
                                stop=(k == KC - 1),
                            )
                        osl = ot[:, m, off : off + ns]
                        if act == "silu":
                            nc.scalar.activation(
                                osl,
                                ps,
                                mybir.ActivationFunctionType.Silu,
                                bias=bt[:, m : m + 1],
                            )
                        else:
                            # CoreSim has no Silu: Identity+Sigmoid+mul
                            yt = opool.tile([P, ns], f32, tag="yt", name="yt")
                            nc.scalar.activation(
                                yt,
                                ps,
                                mybir.ActivationFunctionType.Identity,
                                bias=bt[:, m : m + 1],
                            )
                            st = opool.tile([P, ns], f32, tag="st", name="st")
                            nc.scalar.activation(
                                st,
                                ps,
                                mybir.ActivationFunctionType.Sigmoid,
                                bias=bt[:, m : m + 1],
                            )
                            nc.vector.tensor_mul(osl, yt, st)
                # Output stores ride the (otherwise idle) GpSimd SWDGE ring:
                # no engine's DMA queue head-of-line-blocks another's. The
                # last two (small) tiles' stores use the scalar HWDGE ring,
                # free by then and ~1us lower completion latency.
                out_eng = nc.scalar if (n0, nt) in tiles[-2:] else nc.gpsimd
                out_eng.dma_start(out=os_[:, MC * n0 : MC * (n0 + nt)], in_=ot)

    nc.compile()
    return nc


def prepare(inputs: dict) -> tuple:
    x = np.ascontiguousarray(np.asarray(inputs["x"], dtype=np.float32))
    idx = np.asarray(inputs["expert_indices"]).astype(np.int64)
    ew = np.asarray(inputs["expert_weights"], dtype=np.float32)
    eb = np.asarray(inputs["expert_biases"], dtype=np.float32)
    gw = np.asarray(inputs["bn_weights"], dtype=np.float32)
    gb = np.asarray(inputs["bn_biases"], dtype=np.float32)
    rm = np.asarray(inputs["running_mean"], dtype=np.float32)
    rv = np.asarray(inputs["running_var"], dtype=np.float32)

    # Fold inference BN into the expert weight/bias:
    #   y = (x @ W + eb - rm) * gw/sqrt(rv+eps) + gb = x @ (W*s) + (eb-rm)*s + gb
    s = gw / np.sqrt(rv + EPS)
    wf = ew * s[:, None, :]
    bf = (eb - rm) * s + gb

    perms = [np.nonzero(idx == e)[0] for e in range(E)]
    counts = [len(p) for p in perms]
    cap = max(512, -(-max(counts) // P) * P)
    tiles = []
    n0 = 0
    for t in plan_sizes(cap):
        tiles.append((n0, t))
        n0 += t

    in_maps = []
    for e in range(E):
        xT = np.zeros((IN, cap), dtype=np.float16)
        if counts[e]:
            xT[:, : counts[e]] = x[perms[e]].T.astype(np.float16)
        xv = xT.reshape(KC, P, cap)
        xs = np.empty((P, KC * cap), dtype=np.float16)
        for n0, nt in tiles:
            xs[:, KC * n0 : KC * (n0 + nt)] = (
                xv[:, :, n0 : n0 + nt].transpose(1, 0, 2).reshape(P, KC * nt)
            )
        ws = (
            wf[e]
            .astype(np.float16)
            .reshape(KC, P, HID)
            .transpose(1, 0, 2)
            .reshape(P, KC * HID)
        )
        bs = np.ascontiguousarray(bf[e].reshape(MC, P).T)
        in_maps.append({"xs": xs, "ws": np.ascontiguousarray(ws), "bs": bs})
    return cap, tiles, perms, counts, in_maps


def combine(results: list, cap, tiles, perms, counts) -> np.ndarray:
    out = np.empty((B, HID), dtype=np.float32)
    for e in range(E):
        if not counts[e]:
            continue
        ob = results[e]["os"]
        oT = np.empty((HID, cap), dtype=np.float32)
        for n0, nt in tiles:
            oT[:, n0 : n0 + nt] = (
                ob[:, MC * n0 : MC * (n0 + nt)]
                .reshape(P, MC, nt)
                .transpose(1, 0, 2)
                .reshape(HID, nt)
            )
        out[perms[e]] = oT[:, : counts[e]].T
    return out


def kernel(**inputs) -> np.ndarray:
    cap, tiles, perms, counts, in_maps = prepare(inputs)
    nc = build_bass(cap)
    res = run_bass_kernel_spmd(nc, in_maps, core_ids=list(range(NCORES)))
    return combine(res.results, cap, tiles, perms, counts)


# revision 48
# speedup vs baseline: 1.0201x; 1.0201x over previous
"""MoE expert-network kernel for 8 Trainium2 NeuronCores.

Strategy: expert parallelism (E == n_cores == 8). The host dispatches each
token to its expert's core (an all-to-all in numpy), folds the inference-mode
BatchNorm into the expert weights/bias, and each core runs one dense
[cap, 512] @ [512, 512] GEMM fused with bias + SiLU via the activation engine.

All device tensors are laid out host-side as the exact SBUF tile images
(128-partition-major, block-contiguous per token tile) so every DMA is a
plain 2D contiguous copy with multi-KB lines.

Per-core device program (identical on all cores, SPMD):
  inputs : xs [128, KC*cap]  fp16 - token tiles, partition-major blocks
           ws [128, KC*HID]  fp16 - BN-folded weight tile image
           bs [128, MC]      fp32 - BN-folded bias tile image
  output : os [128, MC*cap]  fp16 - silu(x @ W + b), block per token tile
x is shipped fp16 (~2e-4 rel error, halves the dominant stream); the host
scatters the result back into the full [B, 512] fp32 output.
"""

import sys

for _p in ("/opt/trn_rl_repo",):
    if _p not in sys.path:
        sys.path.append(_p)

import numpy as np

import concourse.bass as bass
import concourse.mybir as mybir
import concourse.tile as tile
from concourse import bacc
from concourse.bass_utils import run_bass_kernel_spmd

B = 32768
IN = 512
HID = 512
E = 8
NCORES = 8
EPS = 1e-5
P = 128  # SBUF partitions
NT = 512  # matmul moving-dim chunk (one fp32 PSUM bank)

KC = IN // P  # contraction chunks
MC = HID // P  # output-feature chunks


def plan_sizes(cap: int) -> list:
    """Token-tile sizes: small tiles at the start (fast pipeline ramp) and at
    the end (short final ACT->store tail), 1024-wide tiles in the middle."""
    sizes = []
    rem = cap
    if rem >= 256 + 1024:
        sizes.append(256)
        rem -= 256
    if rem >= 512 + 1024 + 128:
        sizes.append(512)
        rem -= 512
    while rem >= 1024 + 128:
        sizes.append(1024)
        rem -= 1024
    if rem > 128:
        sizes.append(rem - 128)
        rem = 128
    if rem:
        sizes.append(rem)
    return sizes


def build_bass(cap: int, act: str = "silu") -> bass.Bass:
    nc = bacc.Bacc(
        "TRN2",
        target_bir_lowering=False,
        debug=False,
        enable_asserts=False,
        num_devices=NCORES,
    )
    f32 = mybir.dt.float32
    f16 = mybir.dt.float16

    xs = nc.dram_tensor("xs", [P, KC * cap], f16, kind="ExternalInput").ap()
    ws = nc.dram_tensor("ws", [P, KC * HID], f16, kind="ExternalInput").ap()
    bs = nc.dram_tensor("bs", [P, MC], f32, kind="ExternalInput").ap()
    os_ = nc.dram_tensor("os", [P, MC * cap], f16, kind="ExternalOutput").ap()

    tiles = []
    n0 = 0
    for s in plan_sizes(cap):
        tiles.append((n0, s))
        n0 += s

    with tile.TileContext(nc) as tc:
        with (
            tc.tile_pool(name="wpool", bufs=1) as wpool,
            tc.tile_pool(name="xpool", bufs=8) as xpool,
            tc.tile_pool(name="opool", bufs=6) as opool,
            tc.tile_pool(name="pp", bufs=8, space="PSUM") as pp,
        ):
            # Weight/bias loads ride the scalar HWDGE ring, token loads the
            # sync ring: their triggers issue in parallel after the preamble.
            wt = wpool.tile([P, KC, HID], f16, tag="wt", name="wt")
            nc.scalar.dma_start(out=wt, in_=ws)
            bt = wpool.tile([P, MC], f32, tag="bt", name="bt")
            nc.scalar.dma_start(out=bt, in_=bs)

            for n0, nt in tiles:
                xt = xpool.tile([P, KC, nt], f16, tag="xt", name="xt")
                nc.sync.dma_start(out=xt, in_=xs[:, KC * n0 : KC * (n0 + nt)])
                ot = opool.tile([P, MC, nt], f16, tag="ot", name="ot")
                for off in range(0, nt, NT):
                    ns = min(NT, nt - off)
                    for m in range(MC):
                        ps = pp.tile([P, ns], f32, tag="ps", name="ps")
                        for k in range(KC):
                            nc.tensor.matmul(
                                ps,
                                lhsT=wt[:, k, m * P : (m + 1) * P],
                                rhs=xt[:, k, off : off + ns],
                                start=(k == 0),
                                stop=(k == KC - 1),
                            )
                        osl = ot[:, m, off : off + ns]
                        if act == "silu":
                            nc.scalar.activation(
                                osl,
                                ps,
                                mybir.ActivationFunctionType.Silu,
                                bias=bt[:, m : m + 1],
                            )
                        else:
                            # CoreSim has no Silu: Identity+Sigmoid+mul
                            yt = opool.tile([P, ns], f32, tag="yt", name="yt")
                            nc.scalar.activation(
                                yt,
                                ps,
                                mybir.ActivationFunctionType.Identity,
                                bias=bt[:, m : m + 1],
                            )
                            st = opool.tile([P, ns], f32, tag="st", name="st")
                            nc.scalar.activation(
                                st,
                                ps,
                                mybir.ActivationFunctionType.Sigmoid,
                                bias=bt[:, m : m + 1],
                            )
                            nc.vector.tensor_mul(osl, yt, st)
                # Output stores ride the (otherwise idle) GpSimd SWDGE ring:
                # no engine's DMA queue head-of-line-blocks another's. The
                # last two (small) tiles' stores use the scalar HWDGE ring,
                # free by then and ~1us lower completion latency.
                out_eng = nc.scalar if (n0, nt) in tiles[-2:] else nc.gpsimd
                out_eng.dma_start(out=os_[:, MC * n0 : MC * (n0 + nt)], in_=ot)

    nc.compile()
    return nc


def prepare(inputs: dict) -> tuple:
    x = np.ascontiguousarray(np.asarray(inputs["x"], dtype=np.float32))
    idx = np.asarray(inputs["expert_indices"]).astype(np.int64)
    ew = np.asarray(inputs["expert_weights"], dtype=np.float32)
    eb = np.asarray(inputs["expert_biases"], dtype=np.float32)
    gw = np.asarray(inputs["bn_weights"], dtype=np.float32)
    gb = np.asarray(inputs["bn_biases"], dtype=np.float32)
    rm = np.asarray(inputs["running_mean"], dtype=np.float32)
    rv = np.asarray(inputs["running_var"], dtype=np.float32)

    # Fold inference BN into the expert weight/bias:
    #   y = (x @ W + eb - rm) * gw/sqrt(rv+eps) + gb = x @ (W*s) + (eb-rm)*s + gb
    s = gw / np.sqrt(rv + EPS)
    wf = ew * s[:, None, :]
    bf = (eb - rm) * s + gb

    perms = [np.nonzero(idx == e)[0] for e in range(E)]
    counts = [len(p) for p in perms]
    cap = max(512, -(-max(counts) // P) * P)
    tiles = []
    n0 = 0
    for t in plan_sizes(cap):
        tiles.append((n0, t))
        n0 += t

    in_maps = []
    for e in range(E):
        xT = np.zeros((IN, cap), dtype=np.float16)
        if counts[e]:
            xT[:, : counts[e]] = x[perms[e]].T.astype(np.float16)
        xv = xT.reshape(KC, P, cap)
        xs = np.empty((P, KC * cap), dtype=np.float16)
        for n0, nt in tiles:
            xs[:, KC * n0 : KC * (n0 + nt)] = (
                xv[:, :, n0 : n0 + nt].transpose(1, 0, 2).reshape(P, KC * nt)
            )
        ws = (
            wf[e]
            .astype(np.float16)
            .reshape(KC, P, HID)
            .transpose(1, 0, 2)
            .reshape(P, KC * HID)
        )
        bs = np.ascontiguousarray(bf[e].reshape(MC, P).T)
        in_maps.append({"xs": xs, "ws": np.ascontiguousarray(ws), "bs": bs})
    return cap, tiles, perms, counts, in_maps


def combine(results: list, cap, tiles, perms, counts) -> np.ndarray:
    out = np.empty((B, HID), dtype=np.float32)
    for e in range(E):
        if not counts[e]:
            continue
        ob = results[e]["os"]
        oT = np.empty((HID, cap), dtype=np.float32)
        for n0, nt in tiles:
            oT[:, n0 : n0 + nt] = (
                ob[:, MC * n0 : MC * (n0 + nt)]
                .reshape(P, MC, nt)
                .transpose(1, 0, 2)
                .reshape(HID, nt)
            )
        out[perms[e]] = oT[:, : counts[e]].T
    return out


def kernel(**inputs) -> np.ndarray:
    cap, tiles, perms, counts, in_maps = prepare(inputs)
    nc = build_bass(cap)
    res = run_bass_kernel_spmd(nc, in_maps, core_ids=list(range(NCORES)))
    return combine(res.results, cap, tiles, perms, counts)
